# revision 8
# baseline (speedup 1.0000x reference)
"""Differentiable Bezier path renderer on 8 Trainium2 NeuronCores.

Strategy (v2)
-------------
The reference rasterizes M=2048 path edges into a 512x512 soft
winding-number image:

    wind[h, w] = sum_e coeff(e, h) * sigmoid(x_cross(e, h) - w)
    coeff(e,h) = sigmoid(20 t) * sigmoid(20 (1 - t)) * sign(dy_e) * [|dy_e|>=1e-6]

Only (edge, row) pairs with |coeff| >= 1e-4 matter (~35k of 1M); each
pair's sigmoid saturates outside a 36px transition window around its
crossing x_cross.  The host samples the path, enumerates active pairs,
precomputes per pair the sigmoid bias xb = x_cross - off (off a
32-aligned window base), the coefficient cf, and the local row slot rl,
then assigns rows to cores so each of the 8 cores gets 64 rows with an
equal pair load (no collectives).

On device (per core), pairs are packed 128 to a block (partition axis):
  * sigmoid windows (fp16): either ScalarE directly
    (sigmoid(xb - k), bias trick) or DVE/Pool computes arg = xb - k and
    ScalarE applies one batched sigmoid per ~8-block chunk,
  * DVE builds W2[p, r] = (iota_r == rl_p) * cf_p in fp16 (one-hot
    scatter fused with the coefficient),
  * TensorE accumulates W2.T @ SIG directly into a single [64, 512]
    PSUM bank at the block's column offset (start=False over a zeroing
    matmul), and W2.T @ ones into suffix columns [0, g) of a [64, 16]
    PSUM tile, so after all blocks suffix[:, b] already holds the
    saturated-left contribution for column block b.
  * Epilogue per 256-col half: DVE folds wind + suffix (stride-0
    broadcast add), ScalarE writes alpha = sigmoid(4 wind), DMA out.

The device returns only alpha [64, 512]; the host broadcasts the rgb
color channels while reassembling the 8 per-core row sets.
"""

import numpy as np

import concourse.bacc as bacc
import concourse.mybir as mybir
import concourse.tile as tile
from concourse.bass_utils import run_bass_kernel_spmd

H = 512
W = 512
S = 64          # cubic bezier segments
TSAMP = 32      # samples per segment
NCORES = 8
RPC = H // NCORES  # rows per core
ALIGN = 32
NGRP = 14          # window groups 0..13 (off = 32 g), 14 = saturated-pure
WIN = 18.0         # sigmoid saturation half-width
DROP = 1e-4        # |coeff| threshold (error <= ~3e-3 vs 2e-2 budget)
CLAMP_X = 10000.0
DT = mybir.dt.float32
F16 = mybir.dt.float16
AF = mybir.ActivationFunctionType

# per-group window widths: 72 for g<=12, 96 for g=13 (window [416, 512))
GW = [72] * 13 + [96]
# sigmoid work assignment pattern: V = arg on DVE, P = arg on Pool,
# A = direct ScalarE sigmoid (bias trick)
MODE_PAT = ["V", "P", "P", "V", "P", "V", "P", "P", "A", "V"]
CHUNK_MAX = 8   # blocks per batched-sigmoid chunk

_prog_cache = {}


def _host_prep(control_points):
    cp = np.asarray(control_points, dtype=np.float32)
    p0 = cp[0:3 * S:3][:, None, :]
    p1 = cp[1:3 * S:3][:, None, :]
    p2 = cp[2:3 * S:3][:, None, :]
    p3 = cp[3:3 * S + 1:3][:, None, :]
    t = np.linspace(0.0, 1.0, TSAMP, dtype=np.float32)[None, :, None]
    mt = np.float32(1.0) - t
    pts = (mt ** 3) * p0 + 3.0 * (mt ** 2) * t * p1 \
        + 3.0 * mt * (t ** 2) * p2 + (t ** 3) * p3
    path = pts.reshape(-1, 2).astype(np.float32)

    nxt = np.roll(path, -1, axis=0)
    x0 = path[:, 0]
    y0 = path[:, 1]
    dy = nxt[:, 1] - y0
    dxe = nxt[:, 0] - x0
    dys = (dy + np.float32(1e-8)).astype(np.float32)
    recip = (np.float32(1.0) / dys).astype(np.float32)
    sm = (np.sign(dy) * (np.abs(dy) >= np.float32(1e-6))).astype(np.float32)

    TB = np.float32(0.85)
    g1 = y0 + (-TB) * dys
    g2 = y0 + (np.float32(1.0) + TB) * dys
    rlo = np.maximum(np.ceil(np.minimum(g1, g2)), 0.0).astype(np.int64)
    rhi = np.minimum(np.floor(np.maximum(g1, g2)), H - 1).astype(np.int64)
    act = (sm != 0) & (rhi >= rlo)
    eact = np.nonzero(act)[0]
    counts = (rhi[eact] - rlo[eact] + 1).astype(np.int64)
    pair_edge = np.repeat(eact, counts)
    pair_row = np.concatenate(
        [np.arange(rlo[e], rhi[e] + 1, dtype=np.int64) for e in eact]
    ) if len(eact) else np.zeros(0, np.int64)

    def sg(x):
        return (np.float32(1.0) /
                (np.float32(1.0) + np.exp(-x.astype(np.float32))))

    tval = ((pair_row.astype(np.float32) - y0[pair_edge])
            * recip[pair_edge]).astype(np.float32)
    xcv = (x0[pair_edge] + tval * dxe[pair_edge]).astype(np.float32)
    xcv = np.clip(xcv, -CLAMP_X, CLAMP_X)
    cf = (sg(20.0 * tval) * sg(20.0 * (np.float32(1.0) - tval))
          * sm[pair_edge]).astype(np.float32)

    keep = (np.abs(cf) >= np.float32(DROP)) & (xcv > np.float32(-WIN + 1.0))
    pair_row = pair_row[keep]
    xcv = xcv[keep]
    cf = cf[keep]

    pure = xcv >= np.float32(512.0 + WIN)
    grp = np.clip(np.floor((xcv - np.float32(WIN)) / ALIGN), 0,
                  NGRP - 1).astype(np.int64)
    grp[pure] = NGRP
    xb = (xcv - np.float32(ALIGN) * np.minimum(grp, NGRP - 1)
          ).astype(np.float32)

    # Balanced row -> core assignment.  The device block count is
    # sum_g max_c ceil(cnt[c,g]/128), so greedily assign each row (in
    # descending load order) to the core that minimizes that objective,
    # tie-broken by total load.
    rowcnt = np.bincount(pair_row, minlength=H)
    rowgrp = np.zeros((H, NGRP + 1), np.int64)
    np.add.at(rowgrp, (pair_row, grp), 1)
    order = np.argsort(-rowcnt, kind="stable")
    core_rows = [[] for _ in range(NCORES)]
    loads = np.zeros(NCORES, np.int64)
    cnt = np.zeros((NCORES, NGRP + 1), np.int64)
    for r in order:
        best = None
        for c in range(NCORES):
            if len(core_rows[c]) >= RPC:
                continue
            newc = cnt[c] + rowgrp[r]
            nbt = 0
            for g in range(NGRP + 1):
                mx = newc[g]
                for c2 in range(NCORES):
                    if c2 != c and cnt[c2, g] > mx:
                        mx = cnt[c2, g]
                nbt += -(-mx // 128)
            key = (nbt, loads[c])
            if best is None or key < best[0]:
                best = (key, c)
        c = best[1]
        core_rows[c].append(int(r))
        loads[c] += rowcnt[r]
        cnt[c] += rowgrp[r]
    row_core = np.empty(H, np.int64)
    row_loc = np.empty(H, np.int64)
    for c in range(NCORES):
        for i, r in enumerate(core_rows[c]):
            row_core[r] = c
            row_loc[r] = i

    pair_core = row_core[pair_row]
    # blocks per group = max over cores (SPMD: one program for all cores)
    nbg = []
    for g in range(NGRP + 1):
        ns = np.array([((pair_core == c) & (grp == g)).sum()
                       for c in range(NCORES)])
        nbg.append(int(np.ceil(ns.max() / 128.0)))
    NBT = sum(nbg)

    per_core = []
    for c in range(NCORES):
        prm = np.zeros((NBT, 3, 128), np.float32)  # -> [128, 3*NBT] later
        off = 0
        for g in reversed(range(NGRP + 1)):
            nb = nbg[g]
            if nb == 0:
                continue
            idx = np.nonzero((pair_core == c) & (grp == g))[0]
            n = len(idx)
            fl = prm[off:off + nb].reshape(nb * 3, 128)
            # block-major layout: block j rows (3j, 3j+1, 3j+2)
            xbv = np.zeros(nb * 128, np.float32)
            cfv = np.zeros(nb * 128, np.float32)
            rlv = np.zeros(nb * 128, np.float32)
            xbv[:n] = xb[idx]
            cfv[:n] = cf[idx]
            rlv[:n] = row_loc[pair_row[idx]].astype(np.float32)
            fl[0::3] = xbv.reshape(nb, 128)
            fl[1::3] = cfv.reshape(nb, 128)
            fl[2::3] = rlv.reshape(nb, 128)
            off += nb
        packed = prm.reshape(NBT * 3, 128).T
        per_core.append({"params": np.ascontiguousarray(packed)})
    return per_core, core_rows, tuple(nbg)


def _block_list(nbg):
    """Blocks in emission order (descending group)."""
    blocks = []
    for g in reversed(range(NGRP + 1)):
        blocks += [g] * nbg[g]
    return blocks


def _build_program(nbg, repeats=1):
    key = (nbg, repeats)
    if key in _prog_cache:
        return _prog_cache[key]
    blocks = _block_list(nbg)
    NBT = len(blocks)
    nc = bacc.Bacc("TRN2", target_bir_lowering=False, debug=False,
                   num_devices=NCORES)

    pard = nc.dram_tensor("params", [128, 3 * NBT], DT, kind="ExternalInput")
    outd = nc.dram_tensor("alpha", [RPC, W], DT, kind="ExternalOutput")

    # consts (fp16): [0:96) -iota, [96:192) iota, [192:208) ones,
    # [208:272) r64, [272:336) zeros
    cst = np.zeros((128, 336), np.float16)
    cst[:, 0:96] = -np.arange(96, dtype=np.float16)[None, :]
    cst[:, 96:192] = np.arange(96, dtype=np.float16)[None, :]
    cst[:, 192:208] = 1.0
    cst[:, 208:272] = np.arange(64, dtype=np.float16)[None, :]
    cstd = nc.inline_tensor(np.ascontiguousarray(cst), name="cstconst")

    import contextlib

    with tile.TileContext(nc) as tc:
        with (
            tc.tile_pool(name="const", bufs=2) as cpool,
            tc.tile_pool(name="prm", bufs=2) as ppool,
            tc.tile_pool(name="w2", bufs=10) as wpool,
            tc.tile_pool(name="arg", bufs=3) as apool,
            tc.tile_pool(name="sig", bufs=4) as spool,
            tc.tile_pool(name="fold", bufs=4) as fpool,
            tc.tile_pool(name="psum", bufs=2, space="PSUM") as pspool,
            (tc.For_i(0, repeats, 1) if repeats > 1
             else contextlib.nullcontext()),
        ):
            cstt = cpool.tile([128, 336], F16)
            nc.sync.dma_start(cstt[:], cstd[:])
            niota = cstt[:, 0:96]
            iota = cstt[:, 96:192]
            ones16 = cstt[:, 192:208]
            r64 = cstt[:, 208:272]
            z64 = cstt[:, 272:336]

            part = ppool.tile([128, 3 * NBT], DT)
            c1 = min(24, 3 * NBT)
            nc.sync.dma_start(part[:, 0:c1], pard[:, 0:c1])
            if c1 < 3 * NBT:
                nc.sync.dma_start(part[:, c1:], pard[:, c1:])

            wind = pspool.tile([RPC, W], DT, tag="wind")
            rs = pspool.tile([RPC, 16], DT, tag="rs")

            # zero the accumulators
            nc.tensor.matmul(wind[:], z64[:],
                             niota[:, 0:1].broadcast_to((128, W)),
                             start=True, stop=False, skip_group_check=True)
            nc.tensor.matmul(rs[:], z64[:], ones16[:],
                             start=True, stop=False, skip_group_check=True)

            last_wind = [None]
            last_rs = [None]

            def emit_block(j, g, sig_ap):
                xbc = part[:, 3 * j:3 * j + 1]
                cfc = part[:, 3 * j + 1:3 * j + 2]
                rlc = part[:, 3 * j + 2:3 * j + 3]
                w2 = wpool.tile([128, RPC], F16)
                nc.vector.tensor_scalar(
                    w2[:], r64, rlc, cfc,
                    mybir.AluOpType.is_equal, mybir.AluOpType.mult)
                if g <= NGRP - 1:
                    wg = GW[g]
                    o = ALIGN * g
                    last_wind[0] = nc.tensor.matmul(
                        wind[:, o:o + wg], w2[:], sig_ap,
                        start=False, stop=False, skip_group_check=True)
                gw = 16 if g == NGRP else g
                if gw > 0:
                    last_rs[0] = nc.tensor.matmul(
                        rs[:, 0:gw], w2[:], ones16[:, 0:gw],
                        start=False, stop=False, skip_group_check=True)

            # chunk state: pending (j, g, col) entries sharing one argbuf
            pending = []

            def flush():
                if not pending:
                    return
                total = pending[-1][2] + GW[pending[-1][1]]
                ab, sb = pending[-1][3], pending[-1][4]
                nc.scalar.activation(sb[:, 0:total], ab[:, 0:total],
                                     AF.Sigmoid, bias=0.0, scale=1.0)
                for (j, g, col, _, _) in pending:
                    emit_block(j, g, sb[:, col:col + GW[g]])
                pending.clear()

            mi = 0
            halves_done = [False]

            def epilogue_half(h):
                # cols [256h', ...): h=1 -> [256, 512), h=0 -> [0, 256)
                lo = 256 * h
                foldt = fpool.tile([RPC, 256], DT, tag=f"fold{h}")
                rs_sb = fpool.tile([RPC, 8], DT, tag=f"rs_sb{h}")
                nc.scalar.copy(rs_sb[:], rs[:, 8 * h:8 * h + 8])
                src = wind[:, lo:lo + 256].rearrange(
                    "p (c k) -> p c k", k=ALIGN)
                sfx = rs_sb[:].unsqueeze(2).broadcast_to((RPC, 8, ALIGN))
                dst = foldt[:].rearrange("p (c k) -> p c k", k=ALIGN)
                nc.vector.tensor_tensor(dst, src, sfx, mybir.AluOpType.add)
                alphat = fpool.tile([RPC, 256], DT, tag=f"alpha{h}")
                nc.scalar.activation(alphat[:], foldt[:], AF.Sigmoid,
                                     bias=0.0, scale=4.0)
                nc.sync.dma_start(outd[:, lo:lo + 256], alphat[:])

            for bi, g in enumerate(blocks):
                j = bi
                if g == NGRP:
                    emit_block(j, g, None)
                else:
                    mode = MODE_PAT[mi % len(MODE_PAT)]
                    mi += 1
                    wg = GW[g]
                    if mode == "A":
                        sgt = spool.tile([128, 96], F16)
                        nc.scalar.activation(sgt[:, 0:wg], iota[:, 0:wg],
                                             AF.Sigmoid,
                                             bias=part[:, 3 * j:3 * j + 1],
                                             scale=-1.0)
                        emit_block(j, g, sgt[:, 0:wg])
                    else:
                        col = (pending[-1][2] + GW[pending[-1][1]]
                               ) if pending else 0
                        if not pending:
                            ab = apool.tile([128, CHUNK_MAX * 96], F16)
                            sb = spool.tile([128, CHUNK_MAX * 96], F16)
                        else:
                            ab, sb = pending[-1][3], pending[-1][4]
                        eng = nc.vector if mode == "V" else nc.gpsimd
                        eng.tensor_scalar_add(
                            ab[:, col:col + wg], niota[:, 0:wg],
                            part[:, 3 * j:3 * j + 1])
                        pending.append((j, g, col, ab, sb))
                        if len(pending) >= CHUNK_MAX:
                            flush()
                # after finishing all blocks with g >= 6, do half 1
                if not halves_done[0] and (bi + 1 < len(blocks)
                                           and blocks[bi + 1] <= 5):
                    flush()
                    epilogue_half(1)
                    halves_done[0] = True
            flush()
            if not halves_done[0]:
                epilogue_half(1)
            epilogue_half(0)

    nc.compile()
    _prog_cache[key] = nc
    return nc


def _in_maps(per_core):
    return [{"params": per_core[c]["params"]} for c in range(NCORES)]


def kernel(control_points, color):
    per_core, core_rows, nbg = _host_prep(control_points)
    nc = _build_program(nbg)
    res = run_bass_kernel_spmd(nc, _in_maps(per_core),
                               list(range(NCORES)))
    out = np.empty((H, W, 4), np.float32)
    out[:, :, 0:3] = np.asarray(color, np.float32)[None, None, :]
    for c in range(NCORES):
        out[np.asarray(core_rows[c], np.int64), :, 3] = \
            res.results[c]["alpha"]
    return out


# revision 18
# speedup vs baseline: 1.9436x; 1.9436x over previous
"""Differentiable Bezier path renderer on 8 Trainium2 NeuronCores.

Strategy (v2)
-------------
The reference rasterizes M=2048 path edges into a 512x512 soft
winding-number image:

    wind[h, w] = sum_e coeff(e, h) * sigmoid(x_cross(e, h) - w)
    coeff(e,h) = sigmoid(20 t) * sigmoid(20 (1 - t)) * sign(dy_e) * [|dy_e|>=1e-6]

Only (edge, row) pairs with |coeff| >= 1e-4 matter (~35k of 1M); each
pair's sigmoid saturates outside a 36px transition window around its
crossing x_cross.  The host samples the path, enumerates active pairs,
precomputes per pair the sigmoid bias xb = x_cross - off (off a
32-aligned window base), the coefficient cf, and the local row slot rl,
then assigns rows to cores so each of the 8 cores gets 64 rows with an
equal pair load (no collectives).

On device (per core), pairs are packed 128 to a block (partition axis):
  * sigmoid windows (fp16): either ScalarE directly
    (sigmoid(xb - k), bias trick) or DVE/Pool computes arg = xb - k and
    ScalarE applies one batched sigmoid per ~8-block chunk,
  * DVE builds W2[p, r] = (iota_r == rl_p) * cf_p in fp16 (one-hot
    scatter fused with the coefficient),
  * TensorE accumulates W2.T @ SIG directly into a single [64, 512]
    PSUM bank at the block's column offset (start=False over a zeroing
    matmul), and W2.T @ ones into suffix columns [0, g) of a [64, 16]
    PSUM tile, so after all blocks suffix[:, b] already holds the
    saturated-left contribution for column block b.
  * Epilogue per 256-col half: DVE folds wind + suffix (stride-0
    broadcast add), ScalarE writes alpha = sigmoid(4 wind), DMA out.

The device returns only alpha [64, 512]; the host broadcasts the rgb
color channels while reassembling the 8 per-core row sets.
"""

import numpy as np

import concourse.bacc as bacc
import concourse.mybir as mybir
import concourse.tile as tile
from concourse.bass_utils import run_bass_kernel_spmd

H = 512
W = 512
S = 64          # cubic bezier segments
TSAMP = 32      # samples per segment
NCORES = 8
RPC = H // NCORES  # rows per core
ALIGN = 32
NGRP = 14          # window groups 0..13 (off = 32 g), 14 = saturated-pure
WIN = 18.0         # sigmoid saturation half-width
DROP = 1e-4        # |coeff| threshold (error <= ~3e-3 vs 2e-2 budget)
CLAMP_X = 10000.0
DT = mybir.dt.float32
F16 = mybir.dt.float16
AF = mybir.ActivationFunctionType

# per-group window widths: 72 for g<=12, 96 for g=13 (window [416, 512))
GW = [72] * 13 + [96]
# sigmoid work assignment pattern: V = arg on DVE, P = arg on Pool,
# A = direct ScalarE sigmoid (bias trick)
MODE_PAT = ["V", "V", "V", "V", "A"]
CHUNK_MAX = 8   # blocks per batched-sigmoid chunk

_prog_cache = {}


def _host_prep(control_points):
    cp = np.asarray(control_points, dtype=np.float32)
    p0 = cp[0:3 * S:3][:, None, :]
    p1 = cp[1:3 * S:3][:, None, :]
    p2 = cp[2:3 * S:3][:, None, :]
    p3 = cp[3:3 * S + 1:3][:, None, :]
    t = np.linspace(0.0, 1.0, TSAMP, dtype=np.float32)[None, :, None]
    mt = np.float32(1.0) - t
    pts = (mt ** 3) * p0 + 3.0 * (mt ** 2) * t * p1 \
        + 3.0 * mt * (t ** 2) * p2 + (t ** 3) * p3
    path = pts.reshape(-1, 2).astype(np.float32)

    nxt = np.roll(path, -1, axis=0)
    x0 = path[:, 0]
    y0 = path[:, 1]
    dy = nxt[:, 1] - y0
    dxe = nxt[:, 0] - x0
    dys = (dy + np.float32(1e-8)).astype(np.float32)
    recip = (np.float32(1.0) / dys).astype(np.float32)
    sm = (np.sign(dy) * (np.abs(dy) >= np.float32(1e-6))).astype(np.float32)

    TB = np.float32(0.85)
    g1 = y0 + (-TB) * dys
    g2 = y0 + (np.float32(1.0) + TB) * dys
    rlo = np.maximum(np.ceil(np.minimum(g1, g2)), 0.0).astype(np.int64)
    rhi = np.minimum(np.floor(np.maximum(g1, g2)), H - 1).astype(np.int64)
    act = (sm != 0) & (rhi >= rlo)
    eact = np.nonzero(act)[0]
    counts = (rhi[eact] - rlo[eact] + 1).astype(np.int64)
    pair_edge = np.repeat(eact, counts)
    pair_row = np.concatenate(
        [np.arange(rlo[e], rhi[e] + 1, dtype=np.int64) for e in eact]
    ) if len(eact) else np.zeros(0, np.int64)

    def sg(x):
        return (np.float32(1.0) /
                (np.float32(1.0) + np.exp(-x.astype(np.float32))))

    tval = ((pair_row.astype(np.float32) - y0[pair_edge])
            * recip[pair_edge]).astype(np.float32)
    xcv = (x0[pair_edge] + tval * dxe[pair_edge]).astype(np.float32)
    xcv = np.clip(xcv, -CLAMP_X, CLAMP_X)
    cf = (sg(20.0 * tval) * sg(20.0 * (np.float32(1.0) - tval))
          * sm[pair_edge]).astype(np.float32)

    keep = (np.abs(cf) >= np.float32(DROP)) & (xcv > np.float32(-WIN + 1.0))
    pair_row = pair_row[keep]
    xcv = xcv[keep]
    cf = cf[keep]

    pure = xcv >= np.float32(512.0 + WIN)
    grp = np.clip(np.floor((xcv - np.float32(WIN)) / ALIGN), 0,
                  NGRP - 1).astype(np.int64)
    grp[pure] = NGRP
    xb = (xcv - np.float32(ALIGN) * np.minimum(grp, NGRP - 1)
          ).astype(np.float32)

    # Balanced row -> core assignment.  The device block count is
    # sum_g max_c ceil(cnt[c,g]/128), so greedily assign each row (in
    # descending load order) to the core that minimizes that objective,
    # tie-broken by total load.
    rowcnt = np.bincount(pair_row, minlength=H)
    rowgrp = np.zeros((H, NGRP + 1), np.int64)
    np.add.at(rowgrp, (pair_row, grp), 1)
    order = np.argsort(-rowcnt, kind="stable")
    core_rows = [[] for _ in range(NCORES)]
    loads = np.zeros(NCORES, np.int64)
    cnt = np.zeros((NCORES, NGRP + 1), np.int64)
    for r in order:
        best = None
        for c in range(NCORES):
            if len(core_rows[c]) >= RPC:
                continue
            newc = cnt[c] + rowgrp[r]
            nbt = 0
            for g in range(NGRP + 1):
                mx = newc[g]
                for c2 in range(NCORES):
                    if c2 != c and cnt[c2, g] > mx:
                        mx = cnt[c2, g]
                nbt += -(-mx // 128)
            key = (nbt, loads[c])
            if best is None or key < best[0]:
                best = (key, c)
        c = best[1]
        core_rows[c].append(int(r))
        loads[c] += rowcnt[r]
        cnt[c] += rowgrp[r]
    row_core = np.empty(H, np.int64)
    row_loc = np.empty(H, np.int64)
    for c in range(NCORES):
        for i, r in enumerate(core_rows[c]):
            row_core[r] = c
            row_loc[r] = i

    pair_core = row_core[pair_row]
    # blocks per group = max over cores (SPMD: one program for all cores)
    nbg = []
    for g in range(NGRP + 1):
        ns = np.array([((pair_core == c) & (grp == g)).sum()
                       for c in range(NCORES)])
        nbg.append(int(np.ceil(ns.max() / 128.0)))
    NBT = sum(nbg)

    per_core = []
    for c in range(NCORES):
        prm = np.zeros((NBT, 3, 128), np.float32)  # -> [128, 3*NBT] later
        off = 0
        for g in reversed(range(NGRP + 1)):
            nb = nbg[g]
            if nb == 0:
                continue
            idx = np.nonzero((pair_core == c) & (grp == g))[0]
            n = len(idx)
            fl = prm[off:off + nb].reshape(nb * 3, 128)
            # block-major layout: block j rows (3j, 3j+1, 3j+2)
            xbv = np.zeros(nb * 128, np.float32)
            cfv = np.zeros(nb * 128, np.float32)
            rlv = np.zeros(nb * 128, np.float32)
            xbv[:n] = xb[idx]
            cfv[:n] = cf[idx]
            rlv[:n] = row_loc[pair_row[idx]].astype(np.float32)
            fl[0::3] = xbv.reshape(nb, 128)
            fl[1::3] = cfv.reshape(nb, 128)
            fl[2::3] = rlv.reshape(nb, 128)
            off += nb
        packed = prm.reshape(NBT * 3, 128).T
        per_core.append({"params": np.ascontiguousarray(packed)})
    return per_core, core_rows, tuple(nbg)


def _block_list(nbg):
    """Blocks in emission order (descending group)."""
    blocks = []
    for g in reversed(range(NGRP + 1)):
        blocks += [g] * nbg[g]
    return blocks


def _build_program(nbg, repeats=1):
    key = (nbg, repeats)
    if key in _prog_cache:
        return _prog_cache[key]
    blocks = _block_list(nbg)
    NBT = len(blocks)
    nc = bacc.Bacc("TRN2", target_bir_lowering=False, debug=False,
                   num_devices=NCORES)

    pard = nc.dram_tensor("params", [128, 3 * NBT], DT, kind="ExternalInput")
    outd = nc.dram_tensor("alpha", [RPC, W], DT, kind="ExternalOutput")

    # consts (fp16): [0:96) -iota, [96:192) iota, [192:208) ones,
    # [208:272) r64
    cst = np.zeros((128, 272), np.float16)
    cst[:, 0:96] = -np.arange(96, dtype=np.float16)[None, :]
    cst[:, 96:192] = np.arange(96, dtype=np.float16)[None, :]
    cst[:, 192:208] = 1.0
    cst[:, 208:272] = np.arange(64, dtype=np.float16)[None, :]
    cstd = nc.inline_tensor(np.ascontiguousarray(cst), name="cstconst")

    import contextlib

    with tile.TileContext(nc) as tc:
        with (
            tc.tile_pool(name="const", bufs=2) as cpool,
            tc.tile_pool(name="prm", bufs=2) as ppool,
            tc.tile_pool(name="w2", bufs=10) as wpool,
            tc.tile_pool(name="arg", bufs=3) as apool,
            tc.tile_pool(name="sig", bufs=4) as spool,
            tc.tile_pool(name="fold", bufs=8) as fpool,
            tc.tile_pool(name="psum", bufs=2, space="PSUM") as pspool,
            (tc.For_i(0, repeats, 1) if repeats > 1
             else contextlib.nullcontext()),
        ):
            cstt = cpool.tile([128, 272], F16)
            nc.sync.dma_start(cstt[:], cstd[:])
            niota = cstt[:, 0:96]
            iota = cstt[:, 96:192]
            ones16 = cstt[:, 192:208]
            r64 = cstt[:, 208:272]

            part = ppool.tile([128, 3 * NBT], DT)
            c1 = min(24, 3 * NBT)
            nc.sync.dma_start(part[:, 0:c1], pard[:, 0:c1])
            if c1 < 3 * NBT:
                nc.sync.dma_start(part[:, c1:], pard[:, c1:])

            wind = pspool.tile([RPC, W], DT, tag="wind")
            rs = pspool.tile([RPC, 16], DT, tag="rs")

            # preload the sigmoid activation table while the DMAs are in
            # flight (no data dependency: input is a const AP)
            dummy = fpool.tile([128, 1], DT, tag="dummy")
            nc.scalar.activation(dummy[:],
                                 nc.const_aps.scalar_like(0.0, dummy[:]),
                                 AF.Sigmoid, bias=0.0, scale=1.0)
            # zero the accumulators on DVE, also during the DMA wait
            nc.vector.memset(wind[:], 0.0)
            nc.vector.memset(rs[:], 0.0)

            last_wind = [None]
            last_rs = [None]

            def emit_block(j, g, sig_ap):
                xbc = part[:, 3 * j:3 * j + 1]
                cfc = part[:, 3 * j + 1:3 * j + 2]
                rlc = part[:, 3 * j + 2:3 * j + 3]
                w2 = wpool.tile([128, RPC], F16)
                nc.vector.tensor_scalar(
                    w2[:], r64, rlc, cfc,
                    mybir.AluOpType.is_equal, mybir.AluOpType.mult)
                if g <= NGRP - 1:
                    wg = GW[g]
                    o = ALIGN * g
                    last_wind[0] = nc.tensor.matmul(
                        wind[:, o:o + wg], w2[:], sig_ap,
                        start=False, stop=False, skip_group_check=True)
                gw = 16 if g == NGRP else g
                if gw > 0:
                    last_rs[0] = nc.tensor.matmul(
                        rs[:, 0:gw], w2[:], ones16[:, 0:gw],
                        start=False, stop=False, skip_group_check=True)

            # chunk state: pending (j, g, col) entries sharing one argbuf
            pending = []

            def flush():
                if not pending:
                    return
                total = pending[-1][2] + GW[pending[-1][1]]
                ab, sb = pending[-1][3], pending[-1][4]
                nc.scalar.activation(sb[:, 0:total], ab[:, 0:total],
                                     AF.Sigmoid, bias=0.0, scale=1.0)
                for (j, g, col, _, _) in pending:
                    emit_block(j, g, sb[:, col:col + GW[g]])
                pending.clear()

            mi = 0
            halves_done = [0]

            def epilogue_piece(lo, width):
                # fold + alpha + DMA for cols [lo, lo+width), 32 | lo, width
                nb8 = width // ALIGN
                foldt = fpool.tile([RPC, width], DT, tag=f"fold{lo}")
                rs_sb = fpool.tile([RPC, nb8], DT, tag=f"rs_sb{lo}")
                nc.scalar.copy(rs_sb[:], rs[:, lo // ALIGN:lo // ALIGN + nb8])
                src = wind[:, lo:lo + width].rearrange(
                    "p (c k) -> p c k", k=ALIGN)
                sfx = rs_sb[:].unsqueeze(2).broadcast_to((RPC, nb8, ALIGN))
                dst = foldt[:].rearrange("p (c k) -> p c k", k=ALIGN)
                nc.vector.tensor_tensor(dst, src, sfx, mybir.AluOpType.add)
                alphat = fpool.tile([RPC, width], DT, tag=f"alpha{lo}")
                nc.scalar.activation(alphat[:], foldt[:], AF.Sigmoid,
                                     bias=0.0, scale=4.0)
                nc.sync.dma_start(outd[:, lo:lo + width], alphat[:])

            for bi, g in enumerate(blocks):
                j = bi
                if g == NGRP:
                    emit_block(j, g, None)
                else:
                    mode = MODE_PAT[mi % len(MODE_PAT)]
                    if bi >= len(blocks) - 3:
                        mode = "A"  # shortest dependency tail
                    mi += 1
                    wg = GW[g]
                    if mode == "A":
                        sgt = spool.tile([128, 96], F16)
                        nc.scalar.activation(sgt[:, 0:wg], iota[:, 0:wg],
                                             AF.Sigmoid,
                                             bias=part[:, 3 * j:3 * j + 1],
                                             scale=-1.0)
                        emit_block(j, g, sgt[:, 0:wg])
                    else:
                        col = (pending[-1][2] + GW[pending[-1][1]]
                               ) if pending else 0
                        if not pending:
                            ab = apool.tile([128, CHUNK_MAX * 96], F16)
                            sb = spool.tile([128, CHUNK_MAX * 96], F16)
                        else:
                            ab, sb = pending[-1][3], pending[-1][4]
                        eng = nc.vector if mode == "V" else nc.gpsimd
                        eng.tensor_scalar_add(
                            ab[:, col:col + wg], niota[:, 0:wg],
                            part[:, 3 * j:3 * j + 1])
                        pending.append((j, g, col, ab, sb))
                        if len(pending) >= CHUNK_MAX:
                            flush()
                # epilogue pieces as their column ranges complete:
                # [256,512) after g>=6 done, [128,256) after g>=2 done
                nxt = blocks[bi + 1] if bi + 1 < len(blocks) else -1
                if halves_done[0] == 0 and nxt <= 5:
                    flush()
                    epilogue_piece(256, 256)
                    halves_done[0] = 1
                if halves_done[0] == 1 and nxt <= 1:
                    flush()
                    epilogue_piece(128, 128)
                    halves_done[0] = 2
            flush()
            if halves_done[0] == 0:
                epilogue_piece(256, 256)
                halves_done[0] = 1
            if halves_done[0] == 1:
                epilogue_piece(128, 128)
            epilogue_piece(0, 128)

    nc.compile()
    _prog_cache[key] = nc
    return nc


def _in_maps(per_core):
    return [{"params": per_core[c]["params"]} for c in range(NCORES)]


def kernel(control_points, color):
    per_core, core_rows, nbg = _host_prep(control_points)
    nc = _build_program(nbg)
    res = run_bass_kernel_spmd(nc, _in_maps(per_core),
                               list(range(NCORES)))
    out = np.empty((H, W, 4), np.float32)
    out[:, :, 0:3] = np.asarray(color, np.float32)[None, None, :]
    for c in range(NCORES):
        out[np.asarray(core_rows[c], np.int64), :, 3] = \
            res.results[c]["alpha"]
    return out
